# revision 2
# baseline (speedup 1.0000x reference)
"""Bidirectional LSTM (B=64, T=256, D=512, U=500) on 8 Trainium2 NeuronCores.

Sharding: 2 directions x 4 batch-groups -> 16 samples per core, one direction
per core. Backward cores receive time-reversed x from the host, so the device
program is pure SPMD (identical on all 8 cores).

Per-core program:
  Phase 1 (GEMM): xz[t*16+b, 4U] = x @ Wk + b     (f32r matmuls, K=512, M=4096, N=2000)
  Phase 2 (recurrence), 256 steps:
      z = xz[t] + h @ Wr        (f32r matmuls: lhsT = hT chunks [125,16], rhs = Wr)
      i,f,g,o = sigmoid/tanh gate slices (Keras order i,f,g,o)
      c = f*c + i*g ; h = o*tanh(c)
      hT via PE transpose for the next step's matmul
"""

import numpy as np

B, T, D, U = 64, 256, 512, 500
G4 = 4 * U            # 2000
NCORES = 8
BC = B // 4           # 16 samples per core
KCH, KQ = 4, 125      # U = 4 chunks of 125 (recurrent contraction)
DCH = 4               # D = 4 chunks of 128 (input contraction)
NSL = 500             # gate-slice / PSUM-bank width (<=512 fp32)
MT = (T * BC) // 128  # 32 M-tiles of 128 rows in the input GEMM

_CACHE = {}


def _build_program(steps=T):
    import concourse.bass as bass
    import concourse.bacc as bacc
    import concourse.tile as tile
    import concourse.mybir as mybir
    from concourse.masks import make_identity

    dt = mybir.dt
    AF = mybir.ActivationFunctionType
    f32 = dt.float32
    f32r = dt.float32r

    nc = bacc.Bacc("TRN2")

    xT = nc.dram_tensor("xT", [D, T * BC], f32r, kind="ExternalInput")  # (d, t*16+b)
    h0 = nc.dram_tensor("h0", [BC, U], f32, kind="ExternalInput")
    c0 = nc.dram_tensor("c0", [BC, U], f32, kind="ExternalInput")
    Wk = nc.dram_tensor("Wk", [D, G4], f32r, kind="ExternalInput")
    Wr = nc.dram_tensor("Wr", [U, G4], f32r, kind="ExternalInput")
    bv = nc.dram_tensor("b", [G4], f32, kind="ExternalInput")
    y = nc.dram_tensor("y", [T, BC, U], f32r, kind="ExternalOutput")
    xzo = nc.dram_tensor("xzbuf", [T * BC, G4], f32)

    with tile.TileContext(nc) as tc:
        with tc.tile_pool(name="dram", bufs=1, space="DRAM") as dpool, \
             tc.tile_pool(name="persist", bufs=1) as persist:
            xz = xzo

            # Wr chunks stay resident for the whole kernel: chunk k = Wr[125k:125k+125, :]
            wr_sb = persist.tile([KQ, KCH, G4], f32r)
            for k in range(KCH):
                nc.gpsimd.dma_start(wr_sb[:, k, :], Wr[k * KQ:(k + 1) * KQ, :])
            ident_f = persist.tile([BC, BC], f32)
            make_identity(nc, ident_f)
            ident = persist.tile([BC, BC], f32r)
            nc.vector.tensor_copy(ident, ident_f)

            # ---------------- Phase 1: xz = x @ Wk + b ----------------
            with tc.tile_pool(name="gx", bufs=1) as gx, \
                 tc.tile_pool(name="gpsum", bufs=2, space="PSUM") as gps, \
                 tc.tile_pool(name="gout", bufs=3) as gout:
                xT_sb = gx.tile([128, DCH, T * BC], f32r)
                wk_sb = gx.tile([128, DCH, G4], f32r)
                for k in range(DCH):
                    nc.gpsimd.dma_start(xT_sb[:, k, :], xT[k * 128:(k + 1) * 128, :])
                    nc.gpsimd.dma_start(wk_sb[:, k, :], Wk[k * 128:(k + 1) * 128, :])
                b_bc = gx.tile([128, G4], f32)
                bva = bv[:]
                nc.gpsimd.dma_start(
                    b_bc, bass.AP(bva.tensor, bva.offset, [[0, 128], [1, G4]])
                )
                for m in range(MT):
                    ps = gps.tile([128, 4, 512], f32)
                    for n in range(4):
                        for k in range(DCH):
                            nc.tensor.matmul(
                                ps[:, n, 0:NSL],
                                lhsT=xT_sb[:, k, m * 128:(m + 1) * 128],
                                rhs=wk_sb[:, k, n * NSL:(n + 1) * NSL],
                                start=(k == 0),
                                stop=(k == DCH - 1),
                            )
                    so = gout.tile([128, G4], f32)
                    for n in range(4):
                        nc.vector.tensor_add(
                            so[:, n * NSL:(n + 1) * NSL],
                            ps[:, n, 0:NSL],
                            b_bc[:, n * NSL:(n + 1) * NSL],
                        )
                    nc.sync.dma_start(xz[m * 128:(m + 1) * 128, :], so)

            # ---------------- Phase 2: recurrence ----------------
            # Per step: transpose h_{t-1} chunk-wise, interleaved just-in-time
            # with this step's bank-0 matmuls so the PE never idles long
            # enough for HAM to re-throttle. The o-gate/h tail is chunked by
            # 125 units so next step's first transpose starts ~0.6us after
            # the last matmul.
            with tc.tile_pool(name="state", bufs=2) as st, \
                 tc.tile_pool(name="gates", bufs=2) as gt, \
                 tc.tile_pool(name="xzin", bufs=6) as xzp, \
                 tc.tile_pool(name="rpsum", bufs=1, space="PSUM") as rps, \
                 tc.tile_pool(name="tpsum", bufs=2, space="PSUM") as tps:

                h0t = st.tile([BC, U], f32, tag="h0t")
                c_sb = st.tile([BC, U], f32, tag="c")
                nc.sync.dma_start(h0t, h0[:, :])
                nc.sync.dma_start(c_sb, c0[:, :])
                h_sb = st.tile([BC, U], f32r, tag="h")
                nc.vector.tensor_copy(h_sb, h0t)

                for t in range(steps):
                    xzt = xzp.tile([BC, G4], f32)
                    nc.sync.dma_start(xzt, xz[t * BC:(t + 1) * BC, :])
                    xzv = xzt.rearrange("b (n u) -> b n u", n=4)
                    ps = rps.tile([BC, 4, 512], f32)
                    tpt = tps.tile([KQ, KCH, BC], f32r, tag="tp")
                    hT = st.tile([KQ, KCH, BC], f32r, tag="hT")

                    def tr(j):
                        nc.tensor.transpose(
                            tpt[:, j, :], h_sb[:, j * KQ:(j + 1) * KQ], ident
                        )
                        nc.vector.tensor_copy(hT[:, j, :], tpt[:, j, :])

                    def mm(n, k):
                        nc.tensor.matmul(
                            ps[:, n, 0:NSL],
                            lhsT=hT[:, k, :],
                            rhs=wr_sb[:, k, n * NSL:(n + 1) * NSL],
                            start=(k == 0),
                            stop=(k == KCH - 1),
                        )

                    # JIT interleave: transpose chunk j, then bank-0 matmul k
                    tr(0)
                    tr(1)
                    mm(0, 0)
                    tr(2)
                    mm(0, 1)
                    tr(3)
                    mm(0, 2)
                    mm(0, 3)
                    for n in range(1, 4):
                        for k in range(KCH):
                            mm(n, k)

                    s = gt.tile([BC, 4, NSL], f32, tag="s")
                    a = gt.tile([BC, 4, NSL], f32, tag="a")
                    # Keras gate order i, f, g, o (banks 0..3)
                    nc.vector.tensor_add(s[:, 0:2, :], ps[:, 0:2, 0:NSL], xzv[:, 0:2, :])
                    nc.scalar.activation(a[:, 0:2, :], s[:, 0:2, :], AF.Sigmoid)
                    nc.vector.tensor_add(s[:, 2, :], ps[:, 2, 0:NSL], xzv[:, 2, :])
                    nc.scalar.activation(a[:, 2, :], s[:, 2, :], AF.Tanh)

                    t1 = st.tile([BC, U], f32, tag="t1")
                    t2 = st.tile([BC, U], f32, tag="t2")
                    nc.gpsimd.tensor_mul(t1, a[:, 0, :], a[:, 2, :])
                    nc.gpsimd.tensor_mul(t2, a[:, 1, :], c_sb)
                    c_new = st.tile([BC, U], f32, tag="c")
                    nc.gpsimd.tensor_add(c_new, t1, t2)
                    th = st.tile([BC, U], f32, tag="th")
                    nc.scalar.activation(th, c_new, AF.Tanh)

                    # o-gate + h chunked by 125 units: h chunk q unblocks next
                    # step's transpose q immediately
                    h_new = st.tile([BC, U], f32r, tag="h")
                    for q in range(KCH):
                        sl = slice(q * KQ, (q + 1) * KQ)
                        nc.vector.tensor_add(
                            s[:, 3, sl], ps[:, 3, q * KQ:(q + 1) * KQ], xzv[:, 3, sl]
                        )
                        nc.scalar.activation(a[:, 3, sl], s[:, 3, sl], AF.Sigmoid)
                        nc.vector.tensor_mul(h_new[:, sl], a[:, 3, sl], th[:, sl])
                    nc.sync.dma_start(y[t], h_new)
                    h_sb, c_sb = h_new, c_new
    nc.finalize()
    return nc


def _make_in_maps(x, h_f, c_f, h_b, c_b, Wk_f, Wr_f, b_f, Wk_b, Wr_b, b_b):
    x = np.ascontiguousarray(np.asarray(x, np.float32))
    in_maps = []
    for core in range(NCORES):
        d = core // 4           # 0 = forward, 1 = backward
        g = core % 4
        bs = slice(g * BC, (g + 1) * BC)
        xc = x[bs] if d == 0 else x[bs, ::-1]
        # xT[d, t*16+b] = xc[b, t, d]
        xTc = np.ascontiguousarray(xc.transpose(2, 1, 0).reshape(D, T * BC))
        in_maps.append({
            "xT": xTc,
            "h0": np.ascontiguousarray((h_f if d == 0 else h_b)[bs], np.float32),
            "c0": np.ascontiguousarray((c_f if d == 0 else c_b)[bs], np.float32),
            "Wk": np.ascontiguousarray(Wk_f if d == 0 else Wk_b, np.float32),
            "Wr": np.ascontiguousarray(Wr_f if d == 0 else Wr_b, np.float32),
            "b": np.ascontiguousarray(b_f if d == 0 else b_b, np.float32),
        })
    return in_maps


def kernel(x, h_f, c_f, h_b, c_b, Wk_f, Wr_f, b_f, Wk_b, Wr_b, b_b):
    from concourse.bass_utils import run_bass_kernel_spmd

    if "nc" not in _CACHE:
        _CACHE["nc"] = _build_program()
    nc = _CACHE["nc"]
    in_maps = _make_in_maps(x, h_f, c_f, h_b, c_b, Wk_f, Wr_f, b_f, Wk_b, Wr_b, b_b)

    import os
    trace = os.environ.get("BLSTM_TRACE") == "1"
    tmpdir = os.environ.get("BLSTM_TRACE_DIR") or None
    br = run_bass_kernel_spmd(nc, in_maps, list(range(NCORES)), trace=trace, tmpdir=tmpdir)
    _CACHE["exec_time_ns"] = br.exec_time_ns
    _CACHE["br"] = br
    res = br.results

    out = np.empty((B, T, 2 * U), np.float32)
    for core in range(NCORES):
        d = core // 4
        g = core % 4
        yc = res[core]["y"]                    # [T, BC, U]
        yc = np.transpose(yc, (1, 0, 2))       # [BC, T, U]
        bs = slice(g * BC, (g + 1) * BC)
        if d == 0:
            out[bs, :, :U] = yc
        else:
            out[bs, :, U:] = yc[:, ::-1]
    return out



# revision 5
# speedup vs baseline: 1.5363x; 1.5363x over previous
"""Bidirectional LSTM (B=64, T=256, D=512, U=500) on 8 Trainium2 NeuronCores.

Sharding: 2 directions x 4 batch-groups -> 16 samples per core, one direction
per core. Backward cores receive time-reversed x from the host, so the device
program is pure SPMD (identical on all 8 cores).

Per-core program:
  Phase 1 (GEMM): xz[t*16+b, 4U] = x @ Wk + b     (f32r matmuls, K=512, M=4096, N=2000)
  Phase 2 (recurrence), 256 steps. Gate banks are host-permuted to [f,i,g,o]:
      PSUM bank n accumulates xz (via identity-matmul) + h @ Wr chunks
      sig(f,i) merged from PSUM, tanh(g), sig(o) on ScalarE
      c = f*c + i*g ; h = o*tanh(c)
      hT via 4 PE transposes into one PSUM bank + single DVE cast
"""

import numpy as np

B, T, D, U = 64, 256, 512, 500
G4 = 4 * U            # 2000
NCORES = 8
BC = B // 4           # 16 samples per core
KCH, KQ = 4, 125      # U = 4 chunks of 125 (recurrent contraction)
DCH = 4               # D = 4 chunks of 128 (input contraction)
NSL = 500             # gate-slice / PSUM-bank width (<=512 fp32)
MT = (T * BC) // 128  # 32 M-tiles of 128 rows in the input GEMM

_CACHE = {}


def _build_program(steps=T):
    import concourse.bass as bass
    import concourse.bacc as bacc
    import concourse.tile as tile
    import concourse.mybir as mybir
    from concourse.masks import make_identity

    dt = mybir.dt
    AF = mybir.ActivationFunctionType
    f32 = dt.float32
    f32r = dt.float32r

    nc = bacc.Bacc("TRN2")

    xT = nc.dram_tensor("xT", [D, T * BC], f32r, kind="ExternalInput")  # (d, t*16+b)
    h0 = nc.dram_tensor("h0", [BC, U], f32, kind="ExternalInput")
    c0 = nc.dram_tensor("c0", [BC, U], f32, kind="ExternalInput")
    Wk = nc.dram_tensor("Wk", [D, G4], f32r, kind="ExternalInput")   # cols [f,i,g,o]
    Wr = nc.dram_tensor("Wr", [U, G4], f32r, kind="ExternalInput")   # cols [f,i,g,o]
    bv = nc.dram_tensor("b", [G4], f32, kind="ExternalInput")
    y = nc.dram_tensor("y", [T, BC, U], f32r, kind="ExternalOutput")
    xz = nc.dram_tensor("xzbuf", [T * BC, G4], f32r)

    with tile.TileContext(nc) as tc:
        with tc.tile_pool(name="persist", bufs=1) as persist:
            # Wr chunks stay resident: chunk k = Wr[125k:125k+125, :]
            wr_sb = persist.tile([KQ, KCH, G4], f32r)
            for k in range(KCH):
                nc.gpsimd.dma_start(wr_sb[:, k, :], Wr[k * KQ:(k + 1) * KQ, :])
            ident_f = persist.tile([BC, BC], f32)
            make_identity(nc, ident_f)
            ident = persist.tile([BC, BC], f32r)
            nc.vector.tensor_copy(ident, ident_f)

            # ---------------- Phase 1: xz = x @ Wk + b ----------------
            with tc.tile_pool(name="gx", bufs=1) as gx, \
                 tc.tile_pool(name="gpsum", bufs=2, space="PSUM") as gps, \
                 tc.tile_pool(name="gout", bufs=3) as gout:
                xT_sb = gx.tile([128, DCH, T * BC], f32r)
                wk_sb = gx.tile([128, DCH, G4], f32r)
                for k in range(DCH):
                    nc.gpsimd.dma_start(xT_sb[:, k, :], xT[k * 128:(k + 1) * 128, :])
                    nc.gpsimd.dma_start(wk_sb[:, k, :], Wk[k * 128:(k + 1) * 128, :])
                b_bc = gx.tile([128, G4], f32)
                bva = bv[:]
                nc.gpsimd.dma_start(
                    b_bc, bass.AP(bva.tensor, bva.offset, [[0, 128], [1, G4]])
                )
                for m in range(MT):
                    ps = gps.tile([128, 4, 512], f32)
                    for n in range(4):
                        for k in range(DCH):
                            nc.tensor.matmul(
                                ps[:, n, 0:NSL],
                                lhsT=xT_sb[:, k, m * 128:(m + 1) * 128],
                                rhs=wk_sb[:, k, n * NSL:(n + 1) * NSL],
                                start=(k == 0),
                                stop=(k == DCH - 1),
                            )
                    so = gout.tile([128, G4], f32r)
                    for n in range(4):
                        nc.vector.tensor_add(
                            so[:, n * NSL:(n + 1) * NSL],
                            ps[:, n, 0:NSL],
                            b_bc[:, n * NSL:(n + 1) * NSL],
                        )
                    nc.sync.dma_start(xz[m * 128:(m + 1) * 128, :], so)

            # ---------------- Phase 2: recurrence ----------------
            # Gate banks (host-permuted): 0=f 1=i 2=g 3=o.
            # zfi pool: 2 PSUM banks (f,i adjacent -> one merged sigmoid).
            with tc.tile_pool(name="state", bufs=2) as st, \
                 tc.tile_pool(name="gates", bufs=2) as gt, \
                 tc.tile_pool(name="xzin", bufs=4) as xzp, \
                 tc.tile_pool(name="zfi", bufs=1, space="PSUM") as pfi, \
                 tc.tile_pool(name="zg", bufs=1, space="PSUM") as pg, \
                 tc.tile_pool(name="zo", bufs=1, space="PSUM") as po, \
                 tc.tile_pool(name="tpsum", bufs=2, space="PSUM") as tps:

                c_sb = st.tile([BC, U], f32, tag="c")
                nc.sync.dma_start(c_sb, c0[:, :])
                h0t = st.tile([BC, U], f32, tag="h0t")
                nc.sync.dma_start(h0t, h0[:, :])
                h_prev = st.tile([BC, U], f32r, tag="h")
                nc.vector.tensor_copy(h_prev, h0t)

                # initial hT from h0
                tpt0 = tps.tile([KQ, KCH, BC], f32r, tag="tp")
                for q in range(KCH):
                    nc.tensor.transpose(
                        tpt0[:, q, :], h_prev[:, q * KQ:(q + 1) * KQ], ident
                    )
                hT = st.tile([KQ, KCH, BC], f32r, tag="hT")
                nc.vector.tensor_copy(hT, tpt0)

                # step-0 xz load + accumulate into fresh PSUM tiles
                xzt = xzp.tile([BC, G4], f32r, tag="xz")
                nc.sync.dma_start(xzt, xz[0:BC, :])
                zfi = pfi.tile([BC, 2, 512], f32, tag="zfi")
                zg = pg.tile([BC, 512], f32, tag="zg")
                zo = po.tile([BC, 512], f32, tag="zo")

                def xz_adds(zfi_, zg_, zo_, xzt_):
                    nc.tensor.matmul(zfi_[:, 0, 0:NSL], lhsT=ident,
                                     rhs=xzt_[:, 0:NSL], start=True, stop=False)
                    nc.tensor.matmul(zfi_[:, 1, 0:NSL], lhsT=ident,
                                     rhs=xzt_[:, NSL:2 * NSL], start=True, stop=False)
                    nc.tensor.matmul(zg_[:, 0:NSL], lhsT=ident,
                                     rhs=xzt_[:, 2 * NSL:3 * NSL], start=True, stop=False)
                    nc.tensor.matmul(zo_[:, 0:NSL], lhsT=ident,
                                     rhs=xzt_[:, 3 * NSL:4 * NSL], start=True, stop=False)

                xz_adds(zfi, zg, zo, xzt)

                for t in range(steps):
                    # prefetch next step's xz slice
                    if t + 1 < steps:
                        xzt_n = xzp.tile([BC, G4], f32r, tag="xz")
                        nc.sync.dma_start(
                            xzt_n, xz[(t + 1) * BC:(t + 2) * BC, :])

                    # recurrent matmul burst: (f,i) chunk-major, then g, then o
                    for k in range(KCH):
                        nc.tensor.matmul(
                            zfi[:, 0, 0:NSL], lhsT=hT[:, k, :],
                            rhs=wr_sb[:, k, 0:NSL],
                            start=False, stop=(k == KCH - 1))
                        nc.tensor.matmul(
                            zfi[:, 1, 0:NSL], lhsT=hT[:, k, :],
                            rhs=wr_sb[:, k, NSL:2 * NSL],
                            start=False, stop=(k == KCH - 1))
                    for k in range(KCH):
                        nc.tensor.matmul(
                            zg[:, 0:NSL], lhsT=hT[:, k, :],
                            rhs=wr_sb[:, k, 2 * NSL:3 * NSL],
                            start=False, stop=(k == KCH - 1))
                    for k in range(KCH):
                        nc.tensor.matmul(
                            zo[:, 0:NSL], lhsT=hT[:, k, :],
                            rhs=wr_sb[:, k, 3 * NSL:4 * NSL],
                            start=False, stop=(k == KCH - 1))

                    # next step's xz accumulation (fires as banks free up)
                    if t + 1 < steps:
                        zfi_n = pfi.tile([BC, 2, 512], f32, tag="zfi")
                        zg_n = pg.tile([BC, 512], f32, tag="zg")
                        zo_n = po.tile([BC, 512], f32, tag="zo")
                        xz_adds(zfi_n, zg_n, zo_n, xzt_n)

                    # gates
                    fi = gt.tile([BC, 2, NSL], f32, tag="fi")
                    nc.scalar.activation(fi, zfi[:, :, 0:NSL], AF.Sigmoid)
                    t2 = gt.tile([BC, U], f32, tag="t2")
                    nc.gpsimd.tensor_mul(t2, fi[:, 0, :], c_sb)
                    g_sb = gt.tile([BC, U], f32, tag="g")
                    nc.scalar.activation(g_sb, zg[:, 0:NSL], AF.Tanh)
                    t1 = gt.tile([BC, U], f32, tag="t1")
                    nc.vector.tensor_mul(t1, fi[:, 1, :], g_sb)
                    o_sb = gt.tile([BC, U], f32, tag="o")
                    nc.scalar.activation(o_sb, zo[:, 0:NSL], AF.Sigmoid)
                    c_new = st.tile([BC, U], f32, tag="c")
                    nc.vector.tensor_add(c_new, t1, t2)
                    th = gt.tile([BC, U], f32, tag="th")
                    nc.scalar.activation(th, c_new, AF.Tanh)

                    # h chunks + transposes interleaved
                    h_new = st.tile([BC, U], f32r, tag="h")
                    last = t + 1 >= steps
                    if not last:
                        tpt = tps.tile([KQ, KCH, BC], f32r, tag="tp")
                    for q in range(KCH):
                        sl = slice(q * KQ, (q + 1) * KQ)
                        nc.vector.tensor_mul(h_new[:, sl], o_sb[:, sl], th[:, sl])
                        if not last:
                            nc.tensor.transpose(tpt[:, q, :], h_new[:, sl], ident)
                    if not last:
                        hT_n = st.tile([KQ, KCH, BC], f32r, tag="hT")
                        nc.vector.tensor_copy(hT_n, tpt)
                    nc.sync.dma_start(y[t], h_new)

                    c_sb = c_new
                    if not last:
                        hT = hT_n
                        zfi, zg, zo = zfi_n, zg_n, zo_n
    nc.finalize()
    return nc


# Keras gate order in the weights is [i, f, g, o]; kernel wants [f, i, g, o].
_PERM = np.concatenate([
    np.arange(U, 2 * U),      # f
    np.arange(0, U),          # i
    np.arange(2 * U, 3 * U),  # g
    np.arange(3 * U, 4 * U),  # o
])


def _make_in_maps(x, h_f, c_f, h_b, c_b, Wk_f, Wr_f, b_f, Wk_b, Wr_b, b_b):
    x = np.ascontiguousarray(np.asarray(x, np.float32))
    Wks = [np.ascontiguousarray(np.asarray(Wk_f, np.float32)[:, _PERM]),
           np.ascontiguousarray(np.asarray(Wk_b, np.float32)[:, _PERM])]
    Wrs = [np.ascontiguousarray(np.asarray(Wr_f, np.float32)[:, _PERM]),
           np.ascontiguousarray(np.asarray(Wr_b, np.float32)[:, _PERM])]
    bs = [np.ascontiguousarray(np.asarray(b_f, np.float32)[_PERM]),
          np.ascontiguousarray(np.asarray(b_b, np.float32)[_PERM])]
    in_maps = []
    for core in range(NCORES):
        d = core // 4           # 0 = forward, 1 = backward
        g = core % 4
        bsl = slice(g * BC, (g + 1) * BC)
        xc = x[bsl] if d == 0 else x[bsl, ::-1]
        # xT[d, t*16+b] = xc[b, t, d]
        xTc = np.ascontiguousarray(xc.transpose(2, 1, 0).reshape(D, T * BC))
        in_maps.append({
            "xT": xTc,
            "h0": np.ascontiguousarray((h_f if d == 0 else h_b)[bsl], np.float32),
            "c0": np.ascontiguousarray((c_f if d == 0 else c_b)[bsl], np.float32),
            "Wk": Wks[d],
            "Wr": Wrs[d],
            "b": bs[d],
        })
    return in_maps


def kernel(x, h_f, c_f, h_b, c_b, Wk_f, Wr_f, b_f, Wk_b, Wr_b, b_b):
    from concourse.bass_utils import run_bass_kernel_spmd

    if "nc" not in _CACHE:
        _CACHE["nc"] = _build_program()
    nc = _CACHE["nc"]
    in_maps = _make_in_maps(x, h_f, c_f, h_b, c_b, Wk_f, Wr_f, b_f, Wk_b, Wr_b, b_b)

    import os
    trace = os.environ.get("BLSTM_TRACE") == "1"
    tmpdir = os.environ.get("BLSTM_TRACE_DIR") or None
    br = run_bass_kernel_spmd(nc, in_maps, list(range(NCORES)), trace=trace, tmpdir=tmpdir)
    _CACHE["exec_time_ns"] = br.exec_time_ns
    _CACHE["br"] = br
    res = br.results

    out = np.empty((B, T, 2 * U), np.float32)
    for core in range(NCORES):
        d = core // 4
        g = core % 4
        yc = res[core]["y"]                    # [T, BC, U]
        yc = np.transpose(yc, (1, 0, 2))       # [BC, T, U]
        bsl = slice(g * BC, (g + 1) * BC)
        if d == 0:
            out[bsl, :, :U] = yc
        else:
            out[bsl, :, U:] = yc[:, ::-1]
    return out


# revision 7
# speedup vs baseline: 2.0735x; 1.3496x over previous
"""Bidirectional LSTM (B=64, T=256, D=512, U=500) on 8 Trainium2 NeuronCores.

Sharding: 2 directions x 4 batch-groups -> 16 samples per core, one direction
per core. Backward cores receive time-reversed x from the host, so the device
program is pure SPMD (identical on all 8 cores).

Per-core program:
  Phase 1 (GEMM): xz[t*16+b, 4U] = x @ Wk + b     (f32r matmuls, K=512, M=4096, N=2000)
  Phase 2 (recurrence), 256 steps. Gate banks are host-permuted to [f,g,i,o]:
      PSUM bank n accumulates xz (via identity-matmul) + h @ Wr chunks
      sig/tanh read PSUM directly on ScalarE (order: f, g, i, o)
      t2 = f*c (GpSimd), t1 = i*g, c = t1 + t2 (Vector)
      tail in transposed space: o and c are PE-transposed, tanh(cT) runs
      PSUM->SBUF at 125-partition layout, hT = oT * tanh(cT) -> feeds the
      next matmul directly; y is stored transposed and fixed up on host.
"""

import numpy as np

B, T, D, U = 64, 256, 512, 500
G4 = 4 * U            # 2000
NCORES = 8
BC = B // 4           # 16 samples per core
KCH, KQ = 4, 125      # U = 4 chunks of 125 (recurrent contraction)
DCH = 4               # D = 4 chunks of 128 (input contraction)
NSL = 500             # gate-slice / PSUM-bank width (<=512 fp32)
MT = (T * BC) // 128  # 32 M-tiles of 128 rows in the input GEMM

_CACHE = {}


def _build_program(steps=T):
    import concourse.bass as bass
    import concourse.bacc as bacc
    import concourse.tile as tile
    import concourse.mybir as mybir
    from concourse.masks import make_identity

    dt = mybir.dt
    AF = mybir.ActivationFunctionType
    f32 = dt.float32
    f32r = dt.float32r

    nc = bacc.Bacc("TRN2")

    xT = nc.dram_tensor("xT", [D, T * BC], f32r, kind="ExternalInput")  # (d, t*16+b)
    h0 = nc.dram_tensor("h0", [BC, U], f32, kind="ExternalInput")
    c0 = nc.dram_tensor("c0", [BC, U], f32r, kind="ExternalInput")
    Wk = nc.dram_tensor("Wk", [D, G4], f32r, kind="ExternalInput")   # cols [f,g,i,o]
    Wr = nc.dram_tensor("Wr", [U, G4], f32r, kind="ExternalInput")   # cols [f,g,i,o]
    bv = nc.dram_tensor("b", [G4], f32, kind="ExternalInput")
    # transposed output: yT[t, q, k, b] = h_t[b, 125*k + q]
    yT = nc.dram_tensor("yT", [T, KQ, KCH, BC], f32r, kind="ExternalOutput")
    xz = nc.dram_tensor("xzbuf", [T * BC, G4], f32r)

    with tile.TileContext(nc) as tc:
        with tc.tile_pool(name="persist", bufs=1) as persist:
            # Wr chunks stay resident: chunk k = Wr[125k:125k+125, :]
            wr_sb = persist.tile([KQ, KCH, G4], f32r)
            for k in range(KCH):
                nc.gpsimd.dma_start(wr_sb[:, k, :], Wr[k * KQ:(k + 1) * KQ, :])
            ident_f = persist.tile([BC, BC], f32)
            make_identity(nc, ident_f)
            ident = persist.tile([BC, BC], f32r)
            nc.vector.tensor_copy(ident, ident_f)

            # ---------------- Phase 1: xz = x @ Wk + b ----------------
            with tc.tile_pool(name="gx", bufs=1) as gx, \
                 tc.tile_pool(name="gpsum", bufs=2, space="PSUM") as gps, \
                 tc.tile_pool(name="gout", bufs=3) as gout:
                xT_sb = gx.tile([128, DCH, T * BC], f32r)
                wk_sb = gx.tile([128, DCH, G4], f32r)
                for k in range(DCH):
                    nc.gpsimd.dma_start(xT_sb[:, k, :], xT[k * 128:(k + 1) * 128, :])
                    nc.gpsimd.dma_start(wk_sb[:, k, :], Wk[k * 128:(k + 1) * 128, :])
                b_bc = gx.tile([128, G4], f32)
                bva = bv[:]
                nc.gpsimd.dma_start(
                    b_bc, bass.AP(bva.tensor, bva.offset, [[0, 128], [1, G4]])
                )
                for m in range(MT):
                    ps = gps.tile([128, 4, 512], f32)
                    for n in range(4):
                        for k in range(DCH):
                            nc.tensor.matmul(
                                ps[:, n, 0:NSL],
                                lhsT=xT_sb[:, k, m * 128:(m + 1) * 128],
                                rhs=wk_sb[:, k, n * NSL:(n + 1) * NSL],
                                start=(k == 0),
                                stop=(k == DCH - 1),
                            )
                    so = gout.tile([128, G4], f32r)
                    for n in range(4):
                        nc.vector.tensor_add(
                            so[:, n * NSL:(n + 1) * NSL],
                            ps[:, n, 0:NSL],
                            b_bc[:, n * NSL:(n + 1) * NSL],
                        )
                    nc.sync.dma_start(xz[m * 128:(m + 1) * 128, :], so)

            # ---------------- Phase 2: recurrence ----------------
            # Gate banks (host-permuted): 0=f 1=g 2=i 3=o.
            with tc.tile_pool(name="state", bufs=2) as st, \
                 tc.tile_pool(name="gates", bufs=2) as gt, \
                 tc.tile_pool(name="xzin", bufs=4) as xzp, \
                 tc.tile_pool(name="zf", bufs=1, space="PSUM") as pf, \
                 tc.tile_pool(name="zg", bufs=1, space="PSUM") as pg, \
                 tc.tile_pool(name="zi", bufs=1, space="PSUM") as pi, \
                 tc.tile_pool(name="zo", bufs=1, space="PSUM") as po, \
                 tc.tile_pool(name="tpo", bufs=1, space="PSUM") as tpo_p, \
                 tc.tile_pool(name="tpc", bufs=1, space="PSUM") as tpc_p:

                c_sb = st.tile([BC, U], f32r, tag="c")
                nc.sync.dma_start(c_sb, c0[:, :])
                h0t = st.tile([BC, U], f32, tag="h0t")
                nc.sync.dma_start(h0t, h0[:, :])
                h_prev = st.tile([BC, U], f32r, tag="h0r")
                nc.vector.tensor_copy(h_prev, h0t)

                # initial hT from h0
                tpt0 = tpo_p.tile([KQ, KCH, BC], f32r, tag="tpo")
                for q in range(KCH):
                    nc.tensor.transpose(
                        tpt0[:, q, :], h_prev[:, q * KQ:(q + 1) * KQ], ident
                    )
                hT = st.tile([KQ, KCH, BC], f32r, tag="hT")
                nc.vector.tensor_copy(hT, tpt0)

                # step-0 xz load + accumulate into fresh PSUM tiles
                xzt = xzp.tile([BC, G4], f32r, tag="xz")
                nc.sync.dma_start(xzt, xz[0:BC, :])
                zf = pf.tile([BC, 512], f32, tag="zf")
                zg = pg.tile([BC, 512], f32, tag="zg")
                zi = pi.tile([BC, 512], f32, tag="zi")
                zo = po.tile([BC, 512], f32, tag="zo")

                def xz_add(z_, xzt_, n):
                    nc.tensor.matmul(z_[:, 0:NSL], lhsT=ident,
                                     rhs=xzt_[:, n * NSL:(n + 1) * NSL],
                                     start=True, stop=False)

                for n, z_ in enumerate((zf, zg, zi, zo)):
                    xz_add(z_, xzt, n)

                for t in range(steps):
                    # prefetch next step's xz slice
                    if t + 1 < steps:
                        xzt_n = xzp.tile([BC, G4], f32r, tag="xz")
                        nc.sync.dma_start(
                            xzt_n, xz[(t + 1) * BC:(t + 2) * BC, :])

                    # recurrent matmul burst, bank-major f, g, i, o
                    for n, z_ in enumerate((zf, zg, zi, zo)):
                        for k in range(KCH):
                            nc.tensor.matmul(
                                z_[:, 0:NSL], lhsT=hT[:, k, :],
                                rhs=wr_sb[:, k, n * NSL:(n + 1) * NSL],
                                start=False, stop=(k == KCH - 1))

                    # next step's xz accumulation for f,g,i (o after transposes)
                    if t + 1 < steps:
                        zf_n = pf.tile([BC, 512], f32, tag="zf")
                        zg_n = pg.tile([BC, 512], f32, tag="zg")
                        zi_n = pi.tile([BC, 512], f32, tag="zi")
                        zo_n = po.tile([BC, 512], f32, tag="zo")
                        xz_add(zf_n, xzt_n, 0)
                        xz_add(zg_n, xzt_n, 1)
                        xz_add(zi_n, xzt_n, 2)

                    # gates (ScalarE queue order: f, g, i, o, tanh(cT))
                    f_sb = gt.tile([BC, U], f32, tag="f")
                    nc.scalar.activation(f_sb, zf[:, 0:NSL], AF.Sigmoid)
                    t2 = gt.tile([BC, U], f32, tag="t2")
                    nc.gpsimd.tensor_mul(t2, f_sb, c_sb)
                    g_sb = gt.tile([BC, U], f32, tag="g")
                    nc.scalar.activation(g_sb, zg[:, 0:NSL], AF.Tanh)
                    i_sb = gt.tile([BC, U], f32, tag="i")
                    nc.scalar.activation(i_sb, zi[:, 0:NSL], AF.Sigmoid)
                    t1 = gt.tile([BC, U], f32, tag="t1")
                    nc.vector.tensor_mul(t1, i_sb, g_sb)
                    o_sb = gt.tile([BC, U], f32r, tag="o")
                    nc.scalar.activation(o_sb, zo[:, 0:NSL], AF.Sigmoid)
                    c_new = st.tile([BC, U], f32r, tag="c")
                    nc.vector.tensor_add(c_new, t1, t2)

                    # transposed tail: oT, cT on PE; tanh(cT); hT = oT * tanh(cT)
                    tpo = tpo_p.tile([KQ, KCH, BC], f32r, tag="tpo")
                    for q in range(KCH):
                        nc.tensor.transpose(
                            tpo[:, q, :], o_sb[:, q * KQ:(q + 1) * KQ], ident)
                    tpc = tpc_p.tile([KQ, KCH, BC], f32r, tag="tpc")
                    for q in range(KCH):
                        nc.tensor.transpose(
                            tpc[:, q, :], c_new[:, q * KQ:(q + 1) * KQ], ident)
                    if t + 1 < steps:
                        xz_add(zo_n, xzt_n, 3)

                    thT = gt.tile([KQ, KCH, BC], f32r, tag="thT")
                    nc.scalar.activation(thT, tpc, AF.Tanh)
                    hT_n = st.tile([KQ, KCH, BC], f32r, tag="hT")
                    nc.vector.tensor_mul(hT_n, tpo, thT)
                    nc.sync.dma_start(yT[t], hT_n)

                    c_sb = c_new
                    hT = hT_n
                    if t + 1 < steps:
                        zf, zg, zi, zo = zf_n, zg_n, zi_n, zo_n
    nc.finalize()
    return nc


# Keras gate order in the weights is [i, f, g, o]; kernel wants [f, g, i, o].
_PERM = np.concatenate([
    np.arange(U, 2 * U),      # f
    np.arange(2 * U, 3 * U),  # g
    np.arange(0, U),          # i
    np.arange(3 * U, 4 * U),  # o
])


def _make_in_maps(x, h_f, c_f, h_b, c_b, Wk_f, Wr_f, b_f, Wk_b, Wr_b, b_b):
    x = np.ascontiguousarray(np.asarray(x, np.float32))
    Wks = [np.ascontiguousarray(np.asarray(Wk_f, np.float32)[:, _PERM]),
           np.ascontiguousarray(np.asarray(Wk_b, np.float32)[:, _PERM])]
    Wrs = [np.ascontiguousarray(np.asarray(Wr_f, np.float32)[:, _PERM]),
           np.ascontiguousarray(np.asarray(Wr_b, np.float32)[:, _PERM])]
    bs = [np.ascontiguousarray(np.asarray(b_f, np.float32)[_PERM]),
          np.ascontiguousarray(np.asarray(b_b, np.float32)[_PERM])]
    in_maps = []
    for core in range(NCORES):
        d = core // 4           # 0 = forward, 1 = backward
        g = core % 4
        bsl = slice(g * BC, (g + 1) * BC)
        xc = x[bsl] if d == 0 else x[bsl, ::-1]
        # xT[d, t*16+b] = xc[b, t, d]
        xTc = np.ascontiguousarray(xc.transpose(2, 1, 0).reshape(D, T * BC))
        in_maps.append({
            "xT": xTc,
            "h0": np.ascontiguousarray((h_f if d == 0 else h_b)[bsl], np.float32),
            "c0": np.ascontiguousarray((c_f if d == 0 else c_b)[bsl], np.float32),
            "Wk": Wks[d],
            "Wr": Wrs[d],
            "b": bs[d],
        })
    return in_maps


def kernel(x, h_f, c_f, h_b, c_b, Wk_f, Wr_f, b_f, Wk_b, Wr_b, b_b):
    from concourse.bass_utils import run_bass_kernel_spmd

    if "nc" not in _CACHE:
        _CACHE["nc"] = _build_program()
    nc = _CACHE["nc"]
    in_maps = _make_in_maps(x, h_f, c_f, h_b, c_b, Wk_f, Wr_f, b_f, Wk_b, Wr_b, b_b)

    import os
    trace = os.environ.get("BLSTM_TRACE") == "1"
    tmpdir = os.environ.get("BLSTM_TRACE_DIR") or None
    br = run_bass_kernel_spmd(nc, in_maps, list(range(NCORES)), trace=trace, tmpdir=tmpdir)
    _CACHE["exec_time_ns"] = br.exec_time_ns
    _CACHE["br"] = br
    res = br.results

    out = np.empty((B, T, 2 * U), np.float32)
    for core in range(NCORES):
        d = core // 4
        g = core % 4
        yc = res[core]["yT"]                   # [T, KQ, KCH, BC]
        # yc[t, q, k, b] = h_t[b, 125*k + q] -> [BC, T, U]
        yc = np.ascontiguousarray(np.transpose(yc, (3, 0, 2, 1))).reshape(BC, T, U)
        bsl = slice(g * BC, (g + 1) * BC)
        if d == 0:
            out[bsl, :, :U] = yc
        else:
            out[bsl, :, U:] = yc[:, ::-1]
    return out


# revision 10
# speedup vs baseline: 2.4888x; 1.2003x over previous
"""Bidirectional LSTM (B=64, T=256, D=512, U=500) on 8 Trainium2 NeuronCores.

Sharding: 2 directions x 4 batch-groups -> 16 samples per core, one direction
per core. Backward cores receive time-reversed x from the host, so the device
program is pure SPMD (identical on all 8 cores).

Per-core program:
  Phase 1 (GEMM): xz[t*16+b, 4U] = x @ Wk + b     (f32r matmuls, K=512, M=4096, N=2000)
  Phase 2 (recurrence), 256 steps. Gate banks are host-permuted to [f,g,i,o]:
      PSUM bank n accumulates xz (via identity-matmul) + h @ Wr chunks
      sig/tanh read PSUM directly on ScalarE (order: f, g, i, o)
      t2 = f*c (GpSimd), t1 = i*g, c = t1 + t2 (Vector)
      tail in transposed space: o and c are PE-transposed, tanh(cT) runs
      PSUM->SBUF at 125-partition layout, hT = oT * tanh(cT) -> feeds the
      next matmul directly; y is stored transposed and fixed up on host.
"""

import numpy as np

B, T, D, U = 64, 256, 512, 500
G4 = 4 * U            # 2000
NCORES = 8
BC = B // 4           # 16 samples per core
KCH, KQ = 4, 125      # U = 4 chunks of 125 (recurrent contraction)
DCH = 4               # D = 4 chunks of 128 (input contraction)
NSL = 500             # gate-slice / PSUM-bank width (<=512 fp32)
MT = (T * BC) // 128  # 32 M-tiles of 128 rows in the input GEMM

_CACHE = {}


def _build_program(steps=T):
    import concourse.bass as bass
    import concourse.bacc as bacc
    import concourse.tile as tile
    import concourse.mybir as mybir
    from concourse.masks import make_identity

    dt = mybir.dt
    AF = mybir.ActivationFunctionType
    f32 = dt.float32
    f32r = dt.float32r

    nc = bacc.Bacc("TRN2")

    xT = nc.dram_tensor("xT", [D, T * BC], f32r, kind="ExternalInput")  # (d, t*16+b)
    h0 = nc.dram_tensor("h0", [BC, U], f32, kind="ExternalInput")
    c0 = nc.dram_tensor("c0", [BC, U], f32r, kind="ExternalInput")
    Wk = nc.dram_tensor("Wk", [D, G4], f32r, kind="ExternalInput")   # cols [f,g,i,o]
    Wr = nc.dram_tensor("Wr", [U, G4], f32r, kind="ExternalInput")   # cols [f,g,i,o]
    bv = nc.dram_tensor("b", [G4], f32, kind="ExternalInput")
    # transposed output halves: yTa[t, q, k, b] = h_t[b, 125*k + q] (k=0,1),
    # yTb same for k=2,3
    yTa = nc.dram_tensor("yTa", [T, KQ, 2, BC], f32r, kind="ExternalOutput")
    yTb = nc.dram_tensor("yTb", [T, KQ, 2, BC], f32r, kind="ExternalOutput")
    xz = nc.dram_tensor("xzbuf", [T * BC, G4], f32r)

    with tile.TileContext(nc) as tc:
        with tc.tile_pool(name="persist", bufs=1) as persist:
            # Wr chunks stay resident: chunk k = Wr[125k:125k+125, :]
            wr_sb = persist.tile([KQ, KCH, G4], f32r)
            for k in range(KCH):
                nc.gpsimd.dma_start(wr_sb[:, k, :], Wr[k * KQ:(k + 1) * KQ, :])
            ident_f = persist.tile([BC, BC], f32)
            make_identity(nc, ident_f)
            ident = persist.tile([BC, BC], f32r)
            nc.vector.tensor_copy(ident, ident_f)

            # ---------------- Phase 1: xz = x @ Wk + b ----------------
            with tc.tile_pool(name="gx", bufs=1) as gx, \
                 tc.tile_pool(name="gpsum", bufs=2, space="PSUM") as gps, \
                 tc.tile_pool(name="gout", bufs=3) as gout:
                xT_sb = gx.tile([128, DCH, T * BC], f32r)
                wk_sb = gx.tile([128, DCH, G4], f32r)
                for k in range(DCH):
                    nc.gpsimd.dma_start(xT_sb[:, k, :], xT[k * 128:(k + 1) * 128, :])
                    nc.gpsimd.dma_start(wk_sb[:, k, :], Wk[k * 128:(k + 1) * 128, :])
                b_bc = gx.tile([128, G4], f32)
                bva = bv[:]
                nc.gpsimd.dma_start(
                    b_bc, bass.AP(bva.tensor, bva.offset, [[0, 128], [1, G4]])
                )
                for m in range(MT):
                    ps = gps.tile([128, 4, 512], f32)
                    for n in range(4):
                        for k in range(DCH):
                            nc.tensor.matmul(
                                ps[:, n, 0:NSL],
                                lhsT=xT_sb[:, k, m * 128:(m + 1) * 128],
                                rhs=wk_sb[:, k, n * NSL:(n + 1) * NSL],
                                start=(k == 0),
                                stop=(k == DCH - 1),
                            )
                    so = gout.tile([128, G4], f32r)
                    for n in range(4):
                        nc.vector.tensor_add(
                            so[:, n * NSL:(n + 1) * NSL],
                            ps[:, n, 0:NSL],
                            b_bc[:, n * NSL:(n + 1) * NSL],
                        )
                    nc.sync.dma_start(xz[m * 128:(m + 1) * 128, :], so)

            # ---------------- Phase 2: recurrence ----------------
            # Gate banks (host-permuted): 0=f 1=g 2=i 3=o.  The i bank, c
            # state, cT transpose, tanh(cT) and hT are split into unit-halves
            # A = [0,250) and B = [250,500) so each half pipelines through
            # ScalarE/VectorE/PE independently (separate PSUM banks per half
            # dodge both tile-granular deps and PSUM bank collisions).
            UH = U // 2
            with tc.tile_pool(name="state", bufs=2) as st, \
                 tc.tile_pool(name="gates", bufs=2) as gt, \
                 tc.tile_pool(name="xzin", bufs=4) as xzp, \
                 tc.tile_pool(name="zf", bufs=1, space="PSUM") as pf, \
                 tc.tile_pool(name="zg", bufs=1, space="PSUM") as pg, \
                 tc.tile_pool(name="zia", bufs=1, space="PSUM") as pia, \
                 tc.tile_pool(name="zib", bufs=1, space="PSUM") as pib, \
                 tc.tile_pool(name="zo", bufs=1, space="PSUM") as po, \
                 tc.tile_pool(name="tpo", bufs=1, space="PSUM") as tpo_p, \
                 tc.tile_pool(name="tpca", bufs=1, space="PSUM") as tpca_p, \
                 tc.tile_pool(name="tpcb", bufs=1, space="PSUM") as tpcb_p:

                cA = st.tile([BC, UH], f32r, tag="cA")
                cB = st.tile([BC, UH], f32r, tag="cB")
                nc.sync.dma_start(cA, c0[:, 0:UH])
                nc.sync.dma_start(cB, c0[:, UH:U])
                h0t = st.tile([BC, U], f32, tag="h0t")
                nc.sync.dma_start(h0t, h0[:, :])
                h_prev = st.tile([BC, U], f32r, tag="h0r")
                nc.vector.tensor_copy(h_prev, h0t)

                # initial hT halves from h0
                tpt0 = tpo_p.tile([KQ, KCH, BC], f32r, tag="tpo")
                for q in range(KCH):
                    nc.tensor.transpose(
                        tpt0[:, q, :], h_prev[:, q * KQ:(q + 1) * KQ], ident
                    )
                hTa = st.tile([KQ, 2, BC], f32r, tag="hTa")
                hTb = st.tile([KQ, 2, BC], f32r, tag="hTb")
                nc.vector.tensor_copy(hTa, tpt0[:, 0:2, :])
                nc.vector.tensor_copy(hTb, tpt0[:, 2:4, :])

                # step-0 xz load + accumulate into fresh PSUM tiles
                xzt = xzp.tile([BC, G4], f32r, tag="xz")
                nc.sync.dma_start(xzt, xz[0:BC, :])
                zf = pf.tile([BC, 512], f32, tag="zf")
                zg = pg.tile([BC, 512], f32, tag="zg")
                ziA = pia.tile([BC, 256], f32, tag="ziA")
                ziB = pib.tile([BC, 256], f32, tag="ziB")
                zo = po.tile([BC, 512], f32, tag="zo")

                def xz_add(z_, xzt_, lo, w):
                    nc.tensor.matmul(z_[:, 0:w], lhsT=ident,
                                     rhs=xzt_[:, lo:lo + w],
                                     start=True, stop=False)

                def xz_adds_fgi(zf_, zg_, ziA_, ziB_, xzt_):
                    xz_add(zf_, xzt_, 0, NSL)
                    xz_add(zg_, xzt_, NSL, NSL)
                    xz_add(ziA_, xzt_, 2 * NSL, UH)
                    xz_add(ziB_, xzt_, 2 * NSL + UH, UH)

                xz_adds_fgi(zf, zg, ziA, ziB, xzt)
                xz_add(zo, xzt, 3 * NSL, NSL)

                def rmm(z_, k, col, w, hTh):
                    nc.tensor.matmul(
                        z_[:, 0:w], lhsT=hTh[:, k % 2, :],
                        rhs=wr_sb[:, k, col:col + w],
                        start=False, stop=(k == KCH - 1))

                for t in range(steps):
                    # prefetch next step's xz slice
                    if t + 1 < steps:
                        xzt_n = xzp.tile([BC, G4], f32r, tag="xz")
                        nc.sync.dma_start(
                            xzt_n, xz[(t + 1) * BC:(t + 2) * BC, :])

                    # recurrent matmul burst: f, g (full), i in halves, o
                    for k in range(KCH):
                        rmm(zf, k, 0, NSL, hTa if k < 2 else hTb)
                    for k in range(KCH):
                        rmm(zg, k, NSL, NSL, hTa if k < 2 else hTb)
                    for k in range(KCH):
                        rmm(ziA, k, 2 * NSL, UH, hTa if k < 2 else hTb)
                    for k in range(KCH):
                        rmm(ziB, k, 2 * NSL + UH, UH, hTa if k < 2 else hTb)
                    for k in range(KCH):
                        rmm(zo, k, 3 * NSL, NSL, hTa if k < 2 else hTb)

                    # next step's xz accumulation for f,g,i (o after transposes)
                    if t + 1 < steps:
                        zf_n = pf.tile([BC, 512], f32, tag="zf")
                        zg_n = pg.tile([BC, 512], f32, tag="zg")
                        ziA_n = pia.tile([BC, 256], f32, tag="ziA")
                        ziB_n = pib.tile([BC, 256], f32, tag="ziB")
                        zo_n = po.tile([BC, 512], f32, tag="zo")
                        xz_adds_fgi(zf_n, zg_n, ziA_n, ziB_n, xzt_n)

                    # gates (ScalarE order: f, g, iA, iB, o, tanhA, tanhB)
                    f_sb = gt.tile([BC, U], f32, tag="f")
                    nc.scalar.activation(f_sb, zf[:, 0:NSL], AF.Sigmoid)
                    t2A = gt.tile([BC, UH], f32, tag="t2A")
                    t2B = gt.tile([BC, UH], f32, tag="t2B")
                    nc.gpsimd.tensor_mul(t2A, f_sb[:, 0:UH], cA)
                    nc.gpsimd.tensor_mul(t2B, f_sb[:, UH:U], cB)
                    g_sb = gt.tile([BC, U], f32, tag="g")
                    nc.scalar.activation(g_sb, zg[:, 0:NSL], AF.Tanh)
                    iA_sb = gt.tile([BC, UH], f32, tag="iA")
                    nc.scalar.activation(iA_sb, ziA[:, 0:UH], AF.Sigmoid)
                    t1A = gt.tile([BC, UH], f32, tag="t1A")
                    nc.vector.tensor_mul(t1A, iA_sb, g_sb[:, 0:UH])
                    cA_n = st.tile([BC, UH], f32r, tag="cA")
                    nc.vector.tensor_add(cA_n, t1A, t2A)
                    iB_sb = gt.tile([BC, UH], f32, tag="iB")
                    nc.scalar.activation(iB_sb, ziB[:, 0:UH], AF.Sigmoid)
                    t1B = gt.tile([BC, UH], f32, tag="t1B")
                    nc.vector.tensor_mul(t1B, iB_sb, g_sb[:, UH:U])
                    cB_n = st.tile([BC, UH], f32r, tag="cB")
                    nc.vector.tensor_add(cB_n, t1B, t2B)
                    o_sb = gt.tile([BC, U], f32r, tag="o")
                    nc.scalar.activation(o_sb, zo[:, 0:NSL], AF.Sigmoid)

                    # transposed tail, half-pipelined:
                    # T_cA | T_o | T_cB on PE; tanh(cT half) PSUM->SBUF;
                    # hT half = oT half * tanh(cT half)
                    tpcA = tpca_p.tile([KQ, 2, BC], f32r, tag="tpcA")
                    for q in range(2):
                        nc.tensor.transpose(
                            tpcA[:, q, :], cA_n[:, q * KQ:(q + 1) * KQ], ident)
                    tpo = tpo_p.tile([KQ, KCH, BC], f32r, tag="tpo")
                    for q in range(KCH):
                        nc.tensor.transpose(
                            tpo[:, q, :], o_sb[:, q * KQ:(q + 1) * KQ], ident)
                    tpcB = tpcb_p.tile([KQ, 2, BC], f32r, tag="tpcB")
                    for q in range(2):
                        nc.tensor.transpose(
                            tpcB[:, q, :], cB_n[:, q * KQ:(q + 1) * KQ], ident)
                    if t + 1 < steps:
                        xz_add(zo_n, xzt_n, 3 * NSL, NSL)

                    thTa = gt.tile([KQ, 2, BC], f32r, tag="thTa")
                    nc.scalar.activation(thTa, tpcA, AF.Tanh)
                    hTa_n = st.tile([KQ, 2, BC], f32r, tag="hTa")
                    nc.vector.tensor_mul(hTa_n, tpo[:, 0:2, :], thTa)
                    thTb = gt.tile([KQ, 2, BC], f32r, tag="thTb")
                    nc.scalar.activation(thTb, tpcB, AF.Tanh)
                    hTb_n = st.tile([KQ, 2, BC], f32r, tag="hTb")
                    nc.vector.tensor_mul(hTb_n, tpo[:, 2:4, :], thTb)
                    nc.sync.dma_start(yTa[t], hTa_n)
                    nc.sync.dma_start(yTb[t], hTb_n)

                    cA, cB = cA_n, cB_n
                    hTa, hTb = hTa_n, hTb_n
                    if t + 1 < steps:
                        zf, zg, ziA, ziB, zo = zf_n, zg_n, ziA_n, ziB_n, zo_n
    nc.finalize()
    return nc


# Keras gate order in the weights is [i, f, g, o]; kernel wants [f, g, i, o].
_PERM = np.concatenate([
    np.arange(U, 2 * U),      # f
    np.arange(2 * U, 3 * U),  # g
    np.arange(0, U),          # i
    np.arange(3 * U, 4 * U),  # o
])


def _make_in_maps(x, h_f, c_f, h_b, c_b, Wk_f, Wr_f, b_f, Wk_b, Wr_b, b_b):
    x = np.ascontiguousarray(np.asarray(x, np.float32))
    Wks = [np.ascontiguousarray(np.asarray(Wk_f, np.float32)[:, _PERM]),
           np.ascontiguousarray(np.asarray(Wk_b, np.float32)[:, _PERM])]
    Wrs = [np.ascontiguousarray(np.asarray(Wr_f, np.float32)[:, _PERM]),
           np.ascontiguousarray(np.asarray(Wr_b, np.float32)[:, _PERM])]
    bs = [np.ascontiguousarray(np.asarray(b_f, np.float32)[_PERM]),
          np.ascontiguousarray(np.asarray(b_b, np.float32)[_PERM])]
    in_maps = []
    for core in range(NCORES):
        d = core // 4           # 0 = forward, 1 = backward
        g = core % 4
        bsl = slice(g * BC, (g + 1) * BC)
        xc = x[bsl] if d == 0 else x[bsl, ::-1]
        # xT[d, t*16+b] = xc[b, t, d]
        xTc = np.ascontiguousarray(xc.transpose(2, 1, 0).reshape(D, T * BC))
        in_maps.append({
            "xT": xTc,
            "h0": np.ascontiguousarray((h_f if d == 0 else h_b)[bsl], np.float32),
            "c0": np.ascontiguousarray((c_f if d == 0 else c_b)[bsl], np.float32),
            "Wk": Wks[d],
            "Wr": Wrs[d],
            "b": bs[d],
        })
    return in_maps


def kernel(x, h_f, c_f, h_b, c_b, Wk_f, Wr_f, b_f, Wk_b, Wr_b, b_b):
    from concourse.bass_utils import run_bass_kernel_spmd

    if "nc" not in _CACHE:
        _CACHE["nc"] = _build_program()
    nc = _CACHE["nc"]
    in_maps = _make_in_maps(x, h_f, c_f, h_b, c_b, Wk_f, Wr_f, b_f, Wk_b, Wr_b, b_b)

    import os
    trace = os.environ.get("BLSTM_TRACE") == "1"
    tmpdir = os.environ.get("BLSTM_TRACE_DIR") or None
    br = run_bass_kernel_spmd(nc, in_maps, list(range(NCORES)), trace=trace, tmpdir=tmpdir)
    _CACHE["exec_time_ns"] = br.exec_time_ns
    _CACHE["br"] = br
    res = br.results

    out = np.empty((B, T, 2 * U), np.float32)
    for core in range(NCORES):
        d = core // 4
        g = core % 4
        yc = np.concatenate([res[core]["yTa"], res[core]["yTb"]], axis=2)
        # yc[t, q, k, b] = h_t[b, 125*k + q] -> [BC, T, U]
        yc = np.ascontiguousarray(np.transpose(yc, (3, 0, 2, 1))).reshape(BC, T, U)
        bsl = slice(g * BC, (g + 1) * BC)
        if d == 0:
            out[bsl, :, :U] = yc
        else:
            out[bsl, :, U:] = yc[:, ::-1]
    return out
